# revision 16
# baseline (speedup 1.0000x reference)
"""Local (sliding-window) self-attention Bass kernel for 8 TRN2 NeuronCores.

Problem: B=4, T=4096, C=512, H=8 heads, head_dim=64, window=15.
Sharding: 8 cores = batch(4) x seq-halves(2). Each core processes 2048 query
tokens of one batch element; its x chunk carries a 7-token halo on each side
(zero-padded at sequence edges, matching the reference's jnp.pad semantics),
padded to 2080 rows for DMA alignment.

Per-core dataflow (bf16 matmuls, fp32 PSUM accumulation), transpose-free
attention inner loop:
  x chunk --mask*cast--> xb bf16 --one XBAR DMA per tile--> xT [128,4,2080]
  qT/kT feature-major GEMMs (bias via DVE), v token-major GEMM into
  overlapping 128-row tiles at 114 stride, augmented with a ones column
  per head (v_sb [128, 8, 65]) so AV yields the softmax denominator free.
  Per 114-query block x head:
    scoresT [128k, 114q] = kT.T @ qT  (key-major: exp output feeds AV directly)
    exp on ACT -> band-mask on DVE -> AV matmul -> attnT_unnorm [65,114]
    row 64 = denominator -> reciprocal [1,114] into rden[8,2048]
  Batched normalize: sel [8,128] matmul broadcasts rden over 64 features,
  DVE multiplies aT in place.  proj GEMM + (bias, mask) DVE epilogue.
"""

import math
from contextlib import ExitStack

import ml_dtypes
import numpy as np

import concourse.bacc as bacc
import concourse.bass as bass
import concourse.mybir as mybir
import concourse.tile as tile
from concourse import bass_utils

B, T, C, H, WIN = 4, 4096, 512, 8, 15
D = C // H            # 64
PAD = WIN // 2        # 7
NTOK = T // 2         # 2048 query tokens per core
NKV = 2080            # kv rows per core: 7 + 2048 + 7 = 2062, padded to 2080
QB = 114              # queries per attention block (keys fit 128 partitions)
NQB = 18              # 17 * 114 + 110 = 2048
KCH = [512, 512, 512, 512, 32]  # kv token chunks for feature-major matmuls
SCALE = math.log(WIN) / D
F32 = mybir.dt.float32
BF16 = mybir.dt.bfloat16


def _bandT() -> np.ndarray:
    """[128,114] band: bandT[k, q] = 1 iff q <= k <= q+14 (key-major)."""
    k = np.arange(128)[:, None]
    q = np.arange(QB)[None, :]
    return ((k >= q) & (k <= q + WIN - 1)).astype(ml_dtypes.bfloat16)


def build_program() -> bacc.Bacc:
    nc = bacc.Bacc("TRN2", target_bir_lowering=False, debug=False,
                   enable_asserts=False, num_devices=8)

    xd = nc.dram_tensor("x", [NKV, C], F32, kind="ExternalInput").ap()
    maskd = nc.dram_tensor("mask", [NKV], F32, kind="ExternalInput").ap()
    wqd = nc.dram_tensor("wq", [C, C], F32, kind="ExternalInput").ap()
    bqd = nc.dram_tensor("bq", [C], F32, kind="ExternalInput").ap()
    wkvd = nc.dram_tensor("wkv", [C, 2 * C], F32, kind="ExternalInput").ap()
    bkvd = nc.dram_tensor("bkv", [2 * C], F32, kind="ExternalInput").ap()
    wpd = nc.dram_tensor("wproj", [C, C], F32, kind="ExternalInput").ap()
    bpd = nc.dram_tensor("bproj", [C], F32, kind="ExternalInput").ap()
    bandd = nc.dram_tensor("bandt", [128, QB], BF16, kind="ExternalInput").ap()
    outd = nc.dram_tensor("out", [NTOK, C], F32, kind="ExternalOutput").ap()

    with tile.TileContext(nc) as tc, ExitStack() as ctx:
        sb = ctx.enter_context(tc.tile_pool(name="sb", bufs=1))
        sb_x = ctx.enter_context(tc.tile_pool(name="sb_x", bufs=3))
        sb_a = ctx.enter_context(tc.tile_pool(name="sb_a", bufs=4))
        sb_o = ctx.enter_context(tc.tile_pool(name="sb_o", bufs=3))
        pp_big = ctx.enter_context(tc.tile_pool(name="pp_big", bufs=2, space="PSUM"))
        pp_sc = ctx.enter_context(tc.tile_pool(name="pp_sc", bufs=2, space="PSUM"))
        pp_at = ctx.enter_context(tc.tile_pool(name="pp_at", bufs=2, space="PSUM"))
        pp_bc = ctx.enter_context(tc.tile_pool(name="pp_bc", bufs=2, space="PSUM"))

        # ---- persistent SBUF tensors ----
        xT = sb.tile([128, 4, NKV], BF16, tag="xT", name="xT")
        qT = [sb.tile([128, NTOK], BF16, tag=f"qT{i}", name=f"qT{i}") for i in range(4)]
        kT = [sb.tile([128, NKV], BF16, tag=f"kT{i}", name=f"kT{i}") for i in range(4)]
        v_sb = [sb.tile([128, 8, D + 1], BF16, tag=f"vsb{i}", name=f"vsb{i}")
                for i in range(NQB)]
        aT = [sb.tile([128, NTOK], BF16, tag=f"aT{i}", name=f"aTt{i}") for i in range(4)]
        band = sb.tile([128, QB], BF16, tag="band")
        ones1 = sb.tile([1, D], BF16, tag="ones1")
        wq = [sb.tile([128, C], BF16, tag=f"wq{i}", name=f"wq{i}") for i in range(4)]
        wk = [sb.tile([128, C], BF16, tag=f"wk{i}", name=f"wk{i}") for i in range(4)]
        wv = [sb.tile([128, C], BF16, tag=f"wv{i}", name=f"wv{i}") for i in range(4)]
        wp = [sb.tile([128, C], BF16, tag=f"wp{i}", name=f"wp{i}") for i in range(4)]
        bq_t = sb.tile([128, 4], F32, tag="bq")       # per-partition q bias
        bk_t = sb.tile([128, 4], F32, tag="bk")       # per-partition k bias
        bvB = sb.tile([128, C], F32, tag="bvB")       # v bias bcast over partitions
        bpB = sb.tile([128, C], F32, tag="bpB")       # proj bias bcast
        mq2 = sb.tile([128, NQB], F32, tag="mq2")     # query mask per block

        # ---- constants / weights in (sync queue) ----
        nc.sync.dma_start(band[:], bandd)
        nc.gpsimd.memset(ones1[:], 1.0)
        nc.sync.dma_start(bq_t[:], bqd.rearrange("(a b) -> b a", b=128))
        nc.sync.dma_start(bk_t[:], bkvd[0:C].rearrange("(a b) -> b a", b=128))
        nc.sync.dma_start(bvB[:], bkvd[C:2 * C][None, :].broadcast_to((128, C)))
        nc.sync.dma_start(bpB[:], bpd[None, :].broadcast_to((128, C)))
        nc.sync.dma_start(mq2[0:QB, 0:NQB - 1],
                          maskd[PAD:PAD + QB * (NQB - 1)].rearrange(
                              "(a b) -> b a", b=QB))
        nc.sync.dma_start(mq2[0:NTOK - QB * (NQB - 1), NQB - 1:NQB],
                          maskd[PAD + QB * (NQB - 1):PAD + NTOK][:, None])
        for ci in range(4):
            wqf = sb_x.tile([128, C], F32, tag="wld")
            nc.sync.dma_start(wqf[:], wqd[ci * 128:(ci + 1) * 128, :])
            nc.vector.tensor_copy(wq[ci][:], wqf[:])
            wkf = sb_x.tile([128, 2 * C], F32, tag="wld2")
            nc.sync.dma_start(wkf[:], wkvd[ci * 128:(ci + 1) * 128, :])
            nc.vector.tensor_copy(wk[ci][:], wkf[:, 0:C])
            nc.vector.tensor_copy(wv[ci][:], wkf[:, C:2 * C])
            wpf = sb_x.tile([128, C], F32, tag="wld")
            nc.sync.dma_start(wpf[:], wpd[ci * 128:(ci + 1) * 128, :])
            nc.vector.tensor_copy(wp[ci][:], wpf[:])
        for j in range(NQB):
            nc.gpsimd.memset(v_sb[j][:, :, D:D + 1], 1.0)

        # ---- x in: mask*cast, one 3D XBAR transpose per 128-row tile ----
        for t in range(17):
            r0, r1 = t * 128, min((t + 1) * 128, NKV)
            rows = r1 - r0
            xf = sb_x.tile([128, C], F32, tag="xf")
            nc.gpsimd.dma_start(xf[:rows, :], xd[r0:r1, :])
            mrow = sb_x.tile([128, 1], F32, tag="mrow")
            nc.gpsimd.dma_start(mrow[:rows, :], maskd[r0:r1][:, None])
            xb = sb_x.tile([128, C], BF16, tag="xb")
            nc.vector.tensor_scalar_mul(xb[:rows, :], xf[:rows, :], mrow[:rows, :])
            eng = nc.scalar if t % 2 == 0 else nc.sync
            eng.dma_start_transpose(xT[:, :, r0:r1], xb[:rows, :])

        # ---- qT (feature-major): W stationary, xT moving; bias on DVE ----
        for co in range(4):
            for ch in range(4):
                t0 = ch * 512
                ps = pp_big.tile([128, 512], F32, tag="big")
                for ci in range(4):
                    nc.tensor.matmul(
                        ps[:], wq[ci][:, co * 128:(co + 1) * 128],
                        xT[:, ci, PAD + t0:PAD + t0 + 512],
                        start=(ci == 0), stop=(ci == 3))
                nc.vector.tensor_scalar_add(qT[co][:, t0:t0 + 512], ps[:],
                                            bq_t[:, co:co + 1])

        # ---- kT (feature-major) ----
        for co in range(4):
            t0 = 0
            for w in KCH:
                ps = pp_big.tile([128, 512], F32, tag="big")
                for ci in range(4):
                    nc.tensor.matmul(
                        ps[:, 0:w], wk[ci][:, co * 128:(co + 1) * 128],
                        xT[:, ci, t0:t0 + w],
                        start=(ci == 0), stop=(ci == 3))
                nc.vector.tensor_scalar_add(kT[co][:, t0:t0 + w], ps[:, 0:w],
                                            bk_t[:, co:co + 1])
                t0 += w

        # ---- v (token-major, overlapping 128-row tiles at QB stride) ----
        for j in range(NQB):
            r0 = j * QB
            ps = pp_big.tile([128, 512], F32, tag="big")
            for ci in range(4):
                nc.tensor.matmul(
                    ps[:], xT[:, ci, r0:r0 + 128],
                    wv[ci][:], start=(ci == 0), stop=(ci == 3))
            nc.vector.scalar_tensor_tensor(
                v_sb[j][:, :, 0:D],
                ps.rearrange("p (h c) -> p h c", c=D), 1.0,
                bvB.rearrange("p (h c) -> p h c", c=D),
                op0=mybir.AluOpType.mult, op1=mybir.AluOpType.add)

        # ---- attention: per 114-query block x head, key-major / no transpose ----
        for j in range(NQB):
            qb = min(QB, NTOK - j * QB)
            q0 = j * QB
            for h in range(8):
                hp, hh = divmod(h, 2)
                sc = pp_sc.tile([128, QB], F32, tag="sc")
                nc.tensor.matmul(
                    sc[:, 0:qb],
                    kT[hp][hh * 64:(hh + 1) * 64, q0:q0 + 128],
                    qT[hp][hh * 64:(hh + 1) * 64, q0:q0 + qb],
                    start=True, stop=True)
                al = sb_a.tile([128, QB], BF16, tag="al")
                nc.scalar.activation(al[:, 0:qb], sc[:, 0:qb],
                                     mybir.ActivationFunctionType.Exp,
                                     scale=SCALE)
                alb = sb_a.tile([128, QB], BF16, tag="alb")
                nc.vector.scalar_tensor_tensor(
                    alb[:, 0:qb], al[:, 0:qb], 1.0, band[:, 0:qb],
                    op0=mybir.AluOpType.mult, op1=mybir.AluOpType.mult)
                at = pp_at.tile([D + 1, QB], F32, tag="at")
                nc.tensor.matmul(at[:, 0:qb], v_sb[j][:, h, :], alb[:, 0:qb],
                                 start=True, stop=True)
                rc = sb_a.tile([1, QB], BF16, tag="rc")
                with nc.allow_low_precision("softmax denom reciprocal, bf16 ok"):
                    nc.vector.reciprocal(rc[:, 0:qb], at[D:D + 1, 0:qb])
                bc = pp_bc.tile([D, QB], F32, tag="bc")
                nc.tensor.matmul(bc[:, 0:qb], ones1[:], rc[:, 0:qb],
                                 start=True, stop=True)
                bcs = sb_a.tile([D, QB], BF16, tag="bcs")
                nc.scalar.activation(bcs[:, 0:qb], bc[:, 0:qb],
                                     mybir.ActivationFunctionType.Copy)
                nc.vector.scalar_tensor_tensor(
                    aT[hp][hh * 64:(hh + 1) * 64, q0:q0 + qb],
                    at[0:D, 0:qb], 1.0, bcs[:, 0:qb],
                    op0=mybir.AluOpType.mult, op1=mybir.AluOpType.mult)

        # ---- proj (token-major): attnT stationary, Wproj moving ----
        for j in range(NQB):
            qb = min(QB, NTOK - j * QB)
            q0 = j * QB
            ps = pp_big.tile([128, 512], F32, tag="big")
            for ci in range(4):
                nc.tensor.matmul(
                    ps[0:qb, :], aT[ci][:, q0:q0 + qb],
                    wp[ci][:], start=(ci == 0), stop=(ci == 3))
            ot = sb_o.tile([128, C], F32, tag="ot")
            nc.vector.scalar_tensor_tensor(
                ot[0:qb, :], ps[0:qb, :], 1.0, bpB[0:qb, :],
                op0=mybir.AluOpType.mult, op1=mybir.AluOpType.add)
            nc.vector.tensor_scalar_mul(ot[0:qb, :], ot[0:qb, :],
                                        mq2[0:qb, j:j + 1])
            nc.sync.dma_start(outd[q0:q0 + qb, :], ot[0:qb, :])

    nc.compile()
    return nc


_CACHE: dict = {}


def _get_program() -> bacc.Bacc:
    if "nc" not in _CACHE:
        _CACHE["nc"] = build_program()
    return _CACHE["nc"]


def kernel(x, mask, Wq, bq, Wkv, bkv, Wproj, bproj) -> np.ndarray:
    x = np.asarray(x, np.float32)
    mask = np.asarray(mask, np.float32)
    bandt = np.ascontiguousarray(_bandT())
    nc = _get_program()

    in_maps = []
    for core in range(8):
        b, h = divmod(core, 2)
        s = h * NTOK
        xc = np.zeros((NKV, C), np.float32)
        mc = np.zeros((NKV,), np.float32)
        lo, hi = max(0, s - PAD), min(T, s + NTOK + PAD)
        xc[lo - (s - PAD):lo - (s - PAD) + hi - lo] = x[b, lo:hi]
        mc[lo - (s - PAD):lo - (s - PAD) + hi - lo] = mask[b, lo:hi]
        in_maps.append({
            "x": xc, "mask": mc,
            "wq": np.asarray(Wq, np.float32), "bq": np.asarray(bq, np.float32),
            "wkv": np.asarray(Wkv, np.float32), "bkv": np.asarray(bkv, np.float32),
            "wproj": np.asarray(Wproj, np.float32),
            "bproj": np.asarray(bproj, np.float32),
            "bandt": bandt,
        })

    res = bass_utils.run_bass_kernel_spmd(nc, in_maps, core_ids=list(range(8)))
    out = np.empty((B, T, C), np.float32)
    for core in range(8):
        b, h = divmod(core, 2)
        out[b, h * NTOK:(h + 1) * NTOK] = res.results[core]["out"]
    return out


# revision 23
# speedup vs baseline: 1.4374x; 1.4374x over previous
"""Local (sliding-window) self-attention Bass kernel for 8 TRN2 NeuronCores.

Problem: B=4, T=4096, C=512, H=8 heads, head_dim=64, window=15.
Sharding: 8 cores = batch(4) x seq-halves(2). Each core processes 2048 query
tokens of one batch element; its x chunk carries a 7-token halo on each side
(zero-padded at sequence edges, matching the reference's jnp.pad semantics),
padded to 2080 rows for DMA alignment.

Per-core dataflow (bf16 matmuls, fp32 PSUM accumulation), transpose-free
attention inner loop:
  x chunk --mask*cast--> xb bf16 --one XBAR DMA per tile--> xT [128,4,2080]
  qT/kT feature-major GEMMs (bias via DVE), v token-major GEMM into
  overlapping 128-row tiles at 114 stride, augmented with a ones column
  per head (v_sb [128, 8, 65]) so AV yields the softmax denominator free.
  Per 114-query block x head:
    scoresT [128k, 114q] = kT.T @ qT  (key-major: exp output feeds AV directly)
    exp on ACT -> band-mask on DVE -> AV matmul -> attnT_unnorm [65,114]
    row 64 = denominator -> reciprocal [1,114] into rden[8,2048]
  Batched normalize: sel [8,128] matmul broadcasts rden over 64 features,
  DVE multiplies aT in place.  proj GEMM + (bias, mask) DVE epilogue.
"""

import math
from contextlib import ExitStack

import ml_dtypes
import numpy as np

import concourse.bacc as bacc
import concourse.bass as bass
import concourse.mybir as mybir
import concourse.tile as tile
from concourse import bass_utils

B, T, C, H, WIN = 4, 4096, 512, 8, 15
D = C // H            # 64
PAD = WIN // 2        # 7
NTOK = T // 2         # 2048 query tokens per core
NKV = 2080            # kv rows per core: 7 + 2048 + 7 = 2062, padded to 2080
QB = 114              # queries per attention block (keys fit 128 partitions)
NQB = 18              # 17 * 114 + 110 = 2048
KCH = [512, 512, 512, 512, 32]  # kv token chunks for feature-major matmuls
SCALE = math.log(WIN) / D
F32 = mybir.dt.float32
BF16 = mybir.dt.bfloat16


def _bandT() -> np.ndarray:
    """[128,114] band: bandT[k, q] = 1 iff q <= k <= q+14 (key-major)."""
    k = np.arange(128)[:, None]
    q = np.arange(QB)[None, :]
    return ((k >= q) & (k <= q + WIN - 1)).astype(ml_dtypes.bfloat16)


def build_program() -> bacc.Bacc:
    nc = bacc.Bacc("TRN2", target_bir_lowering=False, debug=False,
                   enable_asserts=False, num_devices=8)

    xd = nc.dram_tensor("x", [NKV, C], F32, kind="ExternalInput").ap()
    maskd = nc.dram_tensor("mask", [NKV], F32, kind="ExternalInput").ap()
    wqd = nc.dram_tensor("wq", [C, C], F32, kind="ExternalInput").ap()
    bqd = nc.dram_tensor("bq", [C], F32, kind="ExternalInput").ap()
    wkvd = nc.dram_tensor("wkv", [C, 2 * C], F32, kind="ExternalInput").ap()
    bkvd = nc.dram_tensor("bkv", [2 * C], F32, kind="ExternalInput").ap()
    wpd = nc.dram_tensor("wproj", [C, C], F32, kind="ExternalInput").ap()
    bpd = nc.dram_tensor("bproj", [C], F32, kind="ExternalInput").ap()
    bandd = nc.dram_tensor("bandt", [128, QB], BF16, kind="ExternalInput").ap()
    outd = nc.dram_tensor("out", [NTOK, C], F32, kind="ExternalOutput").ap()

    with tile.TileContext(nc) as tc, ExitStack() as ctx:
        sb = ctx.enter_context(tc.tile_pool(name="sb", bufs=1))
        sb_x = ctx.enter_context(tc.tile_pool(name="sb_x", bufs=3))
        sb_a = ctx.enter_context(tc.tile_pool(name="sb_a", bufs=4))
        sb_o = ctx.enter_context(tc.tile_pool(name="sb_o", bufs=3))
        pp_big = ctx.enter_context(tc.tile_pool(name="pp_big", bufs=3, space="PSUM"))
        pp_sc = ctx.enter_context(tc.tile_pool(name="pp_sc", bufs=2, space="PSUM"))
        pp_at = ctx.enter_context(tc.tile_pool(name="pp_at", bufs=3, space="PSUM"))

        # ---- persistent SBUF tensors ----
        xT = sb.tile([128, 4, NKV], BF16, tag="xT", name="xT")
        qT = [sb.tile([128, NTOK], BF16, tag=f"qT{i}", name=f"qT{i}") for i in range(4)]
        kT = [sb.tile([128, NKV], BF16, tag=f"kT{i}", name=f"kT{i}") for i in range(4)]
        v_sb = [sb.tile([128, 8, D + 1], BF16, tag=f"vsb{i}", name=f"vsb{i}")
                for i in range(NQB)]
        aT3 = sb.tile([128, 4, NQB * 128], BF16, tag="aT3", name="aT3")
        band = sb.tile([128, QB], BF16, tag="band")
        wq = [sb.tile([128, C], BF16, tag=f"wq{i}", name=f"wq{i}") for i in range(4)]
        wk = [sb.tile([128, C], BF16, tag=f"wk{i}", name=f"wk{i}") for i in range(4)]
        wv = [sb.tile([128, C], BF16, tag=f"wv{i}", name=f"wv{i}") for i in range(4)]
        wp = [sb.tile([128, C], BF16, tag=f"wp{i}", name=f"wp{i}") for i in range(4)]
        bq_t = sb.tile([128, 4], F32, tag="bq")       # per-partition q bias
        bk_t = sb.tile([128, 4], F32, tag="bk")       # per-partition k bias
        bvB = sb.tile([128, C], F32, tag="bvB")       # v bias bcast over partitions
        bpB = sb.tile([128, C], F32, tag="bpB")       # proj bias bcast
        mq2 = sb.tile([128, NQB], F32, tag="mq2")     # query mask per block

        # ---- constants / weights in (sync queue) ----
        nc.sync.dma_start(band[:], bandd)
        nc.sync.dma_start(bq_t[:], bqd.rearrange("(a b) -> b a", b=128))
        nc.sync.dma_start(bk_t[:], bkvd[0:C].rearrange("(a b) -> b a", b=128))
        nc.sync.dma_start(bvB[:], bkvd[C:2 * C][None, :].broadcast_to((128, C)))
        nc.sync.dma_start(bpB[:], bpd[None, :].broadcast_to((128, C)))
        nc.sync.dma_start(mq2[0:QB, 0:NQB - 1],
                          maskd[PAD:PAD + QB * (NQB - 1)].rearrange(
                              "(a b) -> b a", b=QB))
        nc.sync.dma_start(mq2[0:NTOK - QB * (NQB - 1), NQB - 1:NQB],
                          maskd[PAD + QB * (NQB - 1):PAD + NTOK][:, None])
        for ci in range(4):
            wqf = sb_x.tile([128, C], F32, tag="wld")
            nc.sync.dma_start(wqf[:], wqd[ci * 128:(ci + 1) * 128, :])
            nc.vector.tensor_copy(wq[ci][:], wqf[:])
            wkf = sb_x.tile([128, 2 * C], F32, tag="wld2")
            nc.sync.dma_start(wkf[:], wkvd[ci * 128:(ci + 1) * 128, :])
            nc.vector.tensor_copy(wk[ci][:], wkf[:, 0:C])
            nc.vector.tensor_copy(wv[ci][:], wkf[:, C:2 * C])
            wpf = sb_x.tile([128, C], F32, tag="wld")
            nc.sync.dma_start(wpf[:], wpd[ci * 128:(ci + 1) * 128, :])
            nc.vector.tensor_copy(wp[ci][:], wpf[:])
        for j in range(NQB):
            nc.gpsimd.memset(v_sb[j][:, :, D:D + 1], 1.0)

        # ---- x in: mask*cast, one 3D XBAR transpose per 128-row tile ----
        for t in range(17):
            r0, r1 = t * 128, min((t + 1) * 128, NKV)
            rows = r1 - r0
            xf = sb_x.tile([128, C], F32, tag="xf")
            nc.gpsimd.dma_start(xf[:rows, :], xd[r0:r1, :])
            mrow = sb_x.tile([128, 1], F32, tag="mrow")
            nc.gpsimd.dma_start(mrow[:rows, :], maskd[r0:r1][:, None])
            xb = sb_x.tile([128, C], BF16, tag="xb")
            nc.vector.tensor_scalar_mul(xb[:rows, :], xf[:rows, :], mrow[:rows, :])
            eng = nc.scalar if t % 2 == 0 else nc.sync
            eng.dma_start_transpose(xT[:, :, r0:r1], xb[:rows, :])

        # ---- qT (feature-major): W stationary, xT moving; bias on DVE ----
        for co in range(4):
            for ch in range(4):
                t0 = ch * 512
                ps = pp_big.tile([128, 512], F32, tag="big")
                for ci in range(4):
                    nc.tensor.matmul(
                        ps[:], wq[ci][:, co * 128:(co + 1) * 128],
                        xT[:, ci, PAD + t0:PAD + t0 + 512],
                        start=(ci == 0), stop=(ci == 3))
                nc.vector.tensor_scalar_add(qT[co][:, t0:t0 + 512], ps[:],
                                            bq_t[:, co:co + 1])

        # ---- kT (feature-major) ----
        for co in range(4):
            t0 = 0
            for w in KCH:
                ps = pp_big.tile([128, 512], F32, tag="big")
                for ci in range(4):
                    nc.tensor.matmul(
                        ps[:, 0:w], wk[ci][:, co * 128:(co + 1) * 128],
                        xT[:, ci, t0:t0 + w],
                        start=(ci == 0), stop=(ci == 3))
                nc.vector.tensor_scalar_add(kT[co][:, t0:t0 + w], ps[:, 0:w],
                                            bk_t[:, co:co + 1])
                t0 += w

        # ---- v (token-major, overlapping 128-row tiles at QB stride) ----
        for j in range(NQB):
            r0 = j * QB
            ps = pp_big.tile([128, 512], F32, tag="big")
            for ci in range(4):
                nc.tensor.matmul(
                    ps[:], xT[:, ci, r0:r0 + 128],
                    wv[ci][:], start=(ci == 0), stop=(ci == 3))
            nc.vector.scalar_tensor_tensor(
                v_sb[j][:, :, 0:D],
                ps.rearrange("p (h c) -> p h c", c=D), 1.0,
                bvB.rearrange("p (h c) -> p h c", c=D),
                op0=mybir.AluOpType.mult, op1=mybir.AluOpType.add)

        # ---- attention: per 114-query block x head, key-major scores ----
        # AV emits token-major att [qb, 65] (col 64 = softmax denominator),
        # so reciprocal + normalize are cheap per-partition DVE ops; one
        # XBAR transpose per block converts to feature-major for proj.
        for j in range(NQB):
            qb = min(QB, NTOK - j * QB)
            q0 = j * QB
            att_tok = sb_a.tile([128, C], BF16, tag="attok")
            nc.gpsimd.memset(att_tok[96:128, :], 0.0)
            for h in range(8):
                hp, hh = divmod(h, 2)
                sc = pp_sc.tile([128, QB], F32, tag="sc")
                nc.tensor.matmul(
                    sc[:, 0:qb],
                    kT[hp][hh * 64:(hh + 1) * 64, q0:q0 + 128],
                    qT[hp][hh * 64:(hh + 1) * 64, q0:q0 + qb],
                    start=True, stop=True)
                al = sb_a.tile([128, QB], BF16, tag="al")
                nc.scalar.activation(al[:, 0:qb], sc[:, 0:qb],
                                     mybir.ActivationFunctionType.Exp,
                                     scale=SCALE)
                alb = sb_a.tile([128, QB], BF16, tag="alb")
                nc.vector.scalar_tensor_tensor(
                    alb[:, 0:qb], al[:, 0:qb], 1.0, band[:, 0:qb],
                    op0=mybir.AluOpType.mult, op1=mybir.AluOpType.mult)
                at = pp_at.tile([QB, D + 1], F32, tag="at")
                nc.tensor.matmul(at[0:qb, :], alb[:, 0:qb], v_sb[j][:, h, :],
                                 start=True, stop=True)
                rdq = sb_a.tile([QB, 1], F32, tag="rdq")
                nc.vector.reciprocal(rdq[0:qb, :], at[0:qb, D:D + 1])
                nc.vector.tensor_scalar_mul(
                    att_tok[0:qb, h * 64:(h + 1) * 64], at[0:qb, 0:D],
                    rdq[0:qb, :])
            eng = nc.scalar if j % 2 == 0 else nc.sync
            eng.dma_start_transpose(aT3[:, :, j * 128:(j + 1) * 128],
                                    att_tok[:, :])

        # ---- proj (token-major): attnT stationary, Wproj moving ----
        for j in range(NQB):
            qb = min(QB, NTOK - j * QB)
            q0 = j * QB
            ps = pp_big.tile([128, 512], F32, tag="big")
            for ci in range(4):
                nc.tensor.matmul(
                    ps[0:qb, :], aT3[:, ci, j * 128:j * 128 + qb],
                    wp[ci][:], start=(ci == 0), stop=(ci == 3))
            ot = sb_o.tile([128, C], F32, tag="ot")
            nc.vector.scalar_tensor_tensor(
                ot[0:qb, :], ps[0:qb, :], 1.0, bpB[0:qb, :],
                op0=mybir.AluOpType.mult, op1=mybir.AluOpType.add)
            nc.vector.tensor_scalar_mul(ot[0:qb, :], ot[0:qb, :],
                                        mq2[0:qb, j:j + 1])
            nc.sync.dma_start(outd[q0:q0 + qb, :], ot[0:qb, :])

    nc.compile()
    return nc


_CACHE: dict = {}


def _get_program() -> bacc.Bacc:
    if "nc" not in _CACHE:
        _CACHE["nc"] = build_program()
    return _CACHE["nc"]


def kernel(x, mask, Wq, bq, Wkv, bkv, Wproj, bproj) -> np.ndarray:
    x = np.asarray(x, np.float32)
    mask = np.asarray(mask, np.float32)
    bandt = np.ascontiguousarray(_bandT())
    nc = _get_program()

    in_maps = []
    for core in range(8):
        b, h = divmod(core, 2)
        s = h * NTOK
        xc = np.zeros((NKV, C), np.float32)
        mc = np.zeros((NKV,), np.float32)
        lo, hi = max(0, s - PAD), min(T, s + NTOK + PAD)
        xc[lo - (s - PAD):lo - (s - PAD) + hi - lo] = x[b, lo:hi]
        mc[lo - (s - PAD):lo - (s - PAD) + hi - lo] = mask[b, lo:hi]
        in_maps.append({
            "x": xc, "mask": mc,
            "wq": np.asarray(Wq, np.float32), "bq": np.asarray(bq, np.float32),
            "wkv": np.asarray(Wkv, np.float32), "bkv": np.asarray(bkv, np.float32),
            "wproj": np.asarray(Wproj, np.float32),
            "bproj": np.asarray(bproj, np.float32),
            "bandt": bandt,
        })

    res = bass_utils.run_bass_kernel_spmd(nc, in_maps, core_ids=list(range(8)))
    out = np.empty((B, T, C), np.float32)
    for core in range(8):
        b, h = divmod(core, 2)
        out[b, h * NTOK:(h + 1) * NTOK] = res.results[core]["out"]
    return out
